# revision 1
# baseline (speedup 1.0000x reference)
"""Cosformer attention Bass kernel for 8 trn2 NeuronCores.

Sharding: core c handles batch c//2, sequence half c%2 (1024 positions x 1
batch = 1024 tokens). Per-head linear-attention state (kv, ksum) is
AllReduce'd between the two cores sharing a batch.

All matmuls run in bf16 with fp32 PSUM accumulation; LN stats and epilogues
in fp32. PHM weights are kron-expanded on host; LN2 affine + final residual
are folded into the output weight (Wo'' = diag(g2) @ (Wo + I)).
"""

import sys

for _p in ('/opt/trn_rl_repo',):
    if _p not in sys.path:
        sys.path.insert(0, _p)

import importlib.util as _ilu
import os

# The image's antenv lacks axon_hooks (needed for trace=True); register ours.
if 'antenv.axon_hooks' not in sys.modules:
    _hp = '/opt/trn_rl_repo/antenv/axon_hooks.py'
    if os.path.exists(_hp):
        _spec = _ilu.spec_from_file_location('antenv.axon_hooks', _hp)
        _mod = _ilu.module_from_spec(_spec)
        _spec.loader.exec_module(_mod)
        sys.modules['antenv.axon_hooks'] = _mod

import numpy as np
import ml_dtypes

import concourse.bass as bass
import concourse.tile as tile
from concourse import bacc, mybir
from concourse.alu_op_type import AluOpType
from concourse.bass_utils import run_bass_kernel_spmd

BF16 = ml_dtypes.bfloat16
FP32 = mybir.dt.float32
BF = mybir.dt.bfloat16

L, N, E, H, D = 2048, 4, 1024, 16, 64
T = 1024            # tokens per core
NT = T // 128       # 8 token tiles
NK = E // 128       # 8 contraction tiles
NJ = E // 128       # 8 output-feature tiles
NCORES = 8
EPS_LN = 1e-5
EPS_ATTN = 1e-6

_BUILD_CACHE = {}


def _build_program(flags):
    """Build the SPMD Bass program. flags: (has_g1b1, has_qb, has_kb, has_vb, has_b2o)."""
    has_g1b1, has_qb, has_kb, has_vb, has_b2o = flags

    nc = bacc.Bacc("TRN2", target_bir_lowering=False, debug=False,
                   num_devices=NCORES)

    # ---- DRAM I/O ----
    d_x_tm = nc.dram_tensor('x_tm', [T, E], FP32, kind='ExternalInput')
    d_x_fm = nc.dram_tensor('x_fm', [E, T], BF, kind='ExternalInput')
    d_wq = nc.dram_tensor('wq', [E, E], BF, kind='ExternalInput')
    d_wk = nc.dram_tensor('wk', [E, E], BF, kind='ExternalInput')
    d_wv = nc.dram_tensor('wv', [E, E], BF, kind='ExternalInput')
    d_wo = nc.dram_tensor('wo2', [E, E], BF, kind='ExternalInput')
    d_sb = nc.dram_tensor('s_bcast', [128, T], BF, kind='ExternalInput')
    d_cb = nc.dram_tensor('c_bcast', [128, T], BF, kind='ExternalInput')
    d_scol = nc.dram_tensor('s_cols', [128, NT], FP32, kind='ExternalInput')
    d_ccol = nc.dram_tensor('c_cols', [128, NT], FP32, kind='ExternalInput')
    d_g1b = nc.dram_tensor('g1_b', [128, E], FP32, kind='ExternalInput') if has_g1b1 else None
    d_b1b = nc.dram_tensor('b1_b', [128, E], FP32, kind='ExternalInput') if has_g1b1 else None
    d_qbc = nc.dram_tensor('qb_cols', [128, NJ], FP32, kind='ExternalInput') if has_qb else None
    d_kbb = nc.dram_tensor('kb_b', [128, E], FP32, kind='ExternalInput') if has_kb else None
    d_vbb = nc.dram_tensor('vb_b', [128, E], FP32, kind='ExternalInput') if has_vb else None
    d_b2ob = nc.dram_tensor('b2o_b', [128, E], FP32, kind='ExternalInput') if has_b2o else None
    d_out = nc.dram_tensor('out', [T, E], FP32, kind='ExternalOutput')

    RG = [[0, 1], [2, 3], [4, 5], [6, 7]]

    with tile.TileContext(nc) as tc:
        with (
            tc.tile_pool(name='persist', bufs=1) as pp,
            tc.tile_pool(name='wpool', bufs=2) as wp,
            tc.tile_pool(name='dram', bufs=1, space='DRAM') as dp,
        ):
            # ---- constants ----
            sbt = pp.tile([128, T], BF, tag='sbt')
            cbt = pp.tile([128, T], BF, tag='cbt')
            scol = pp.tile([128, NT], FP32, tag='scol')
            ccol = pp.tile([128, NT], FP32, tag='ccol')
            eps1 = pp.tile([128, 1], FP32, tag='eps1')
            nc.sync.dma_start(out=sbt, in_=d_sb[:])
            nc.sync.dma_start(out=cbt, in_=d_cb[:])
            nc.sync.dma_start(out=scol, in_=d_scol[:])
            nc.sync.dma_start(out=ccol, in_=d_ccol[:])
            nc.vector.memset(eps1, EPS_LN)
            g1b = b1b = qbc = kbb = vbb = b2ob = None
            if has_g1b1:
                g1b = pp.tile([128, E], FP32, tag='g1b')
                b1b = pp.tile([128, E], FP32, tag='b1b')
                nc.sync.dma_start(out=g1b, in_=d_g1b[:])
                nc.sync.dma_start(out=b1b, in_=d_b1b[:])
            if has_qb:
                qbc = pp.tile([128, NJ], FP32, tag='qbc')
                nc.sync.dma_start(out=qbc, in_=d_qbc[:])
            if has_kb:
                kbb = pp.tile([128, E], FP32, tag='kbb')
                nc.sync.dma_start(out=kbb, in_=d_kbb[:])
            if has_vb:
                vbb = pp.tile([128, E], FP32, tag='vbb')
                nc.sync.dma_start(out=vbb, in_=d_vbb[:])
            if has_b2o:
                b2ob = pp.tile([128, E], FP32, tag='b2ob')
                nc.sync.dma_start(out=b2ob, in_=d_b2ob[:])

            # ---- weights (rotating 2 slots: Wk, Wv, then Wq, Wo2) ----
            wk_t = wp.tile([128, NK, E], BF, tag='W')
            nc.gpsimd.dma_start(out=wk_t, in_=d_wk[:].rearrange('(k p) e -> p k e', p=128))
            wv_t = wp.tile([128, NK, E], BF, tag='W')
            nc.gpsimd.dma_start(out=wv_t, in_=d_wv[:].rearrange('(k p) e -> p k e', p=128))

            # DRAM scratch
            qn_dram = dp.tile([T, E], BF)
            xh_dram = dp.tile([T, E], BF)
            kv_cc_in = dp.tile([128, H * 65], FP32)
            kv_cc_out = dp.tile([128, H * 65], FP32)

            # ================= Phase A: LN1 -> qn (bf16) to DRAM ============
            with tc.tile_pool(name='ln1', bufs=3) as ap:
                for i in range(NT):
                    xt = ap.tile([128, E], FP32, tag='xt')
                    nc.sync.dma_start(out=xt, in_=d_x_tm[i * 128:(i + 1) * 128, :])
                    st = ap.tile([128, 2, 6], FP32, tag='st')
                    xg = xt[:].rearrange('p (g d) -> p g d', g=2)
                    nc.vector.bn_stats(out=st[:, 0, :], in_=xg[:, 0, :])
                    nc.vector.bn_stats(out=st[:, 1, :], in_=xg[:, 1, :])
                    mv = ap.tile([128, 2], FP32, tag='mv')
                    nc.vector.bn_aggr(out=mv, in_=st)
                    rstd = ap.tile([128, 1], FP32, tag='rstd')
                    nc.scalar.activation(out=rstd, in_=mv[:, 1:2],
                                         func=mybir.ActivationFunctionType.Sqrt,
                                         bias=eps1, scale=1.0)
                    nc.vector.reciprocal(out=rstd, in_=rstd)
                    qnt = ap.tile([128, E], BF, tag='qnt')
                    if has_g1b1:
                        tmp = ap.tile([128, E], FP32, tag='qtmp')
                        nc.vector.tensor_scalar(out=tmp, in0=xt, scalar1=mv[:, 0:1],
                                                scalar2=rstd, op0=AluOpType.subtract,
                                                op1=AluOpType.mult)
                        nc.vector.tensor_mul(tmp, tmp, g1b)
                        nc.vector.tensor_tensor(out=qnt, in0=tmp, in1=b1b,
                                                op=AluOpType.add)
                    else:
                        nc.vector.tensor_scalar(out=qnt, in0=xt, scalar1=mv[:, 0:1],
                                                scalar2=rstd, op0=AluOpType.subtract,
                                                op1=AluOpType.mult)
                    nc.gpsimd.dma_start(out=qn_dram[i * 128:(i + 1) * 128, :], in_=qnt)

            with (
                tc.tile_pool(name='bc', bufs=1) as bcp,
            ):
                # ---- feature-major x ----
                xfm = bcp.tile([128, NK, T], BF, tag='xfm')
                nc.sync.dma_start(out=xfm, in_=d_x_fm[:].rearrange('(k p) t -> p k t', p=128))

                ks_all = bcp.tile([128, NT, E], BF, tag='ks')
                kc_all = bcp.tile([128, NT, E], BF, tag='kc')
                v_aug = bcp.tile([128, NT, H, 65], BF, tag='vaug')
                nc.vector.memset(v_aug[:, :, :, 64:65], 1.0)

                # ============ Phase B: k, v matmuls (token-major out) =======
                with tc.tile_pool(name='psB', bufs=4, space='PSUM') as psb:
                    for i in range(NT):
                        for ch in range(2):
                            csl = slice(ch * 512, (ch + 1) * 512)
                            pk = psb.tile([128, 512], FP32, tag='psB')
                            for k in range(NK):
                                nc.tensor.matmul(pk, lhsT=xfm[:, k, i * 128:(i + 1) * 128],
                                                 rhs=wk_t[:, k, csl],
                                                 start=(k == 0), stop=(k == NK - 1))
                            if has_kb:
                                nc.vector.tensor_tensor(out=pk, in0=pk, in1=kbb[:, csl],
                                                        op=AluOpType.add)
                            nc.vector.tensor_scalar(out=ks_all[:, i, csl], in0=pk,
                                                    scalar1=0.0, scalar2=scol[:, i:i + 1],
                                                    op0=AluOpType.max, op1=AluOpType.mult)
                            nc.vector.tensor_scalar(out=kc_all[:, i, csl], in0=pk,
                                                    scalar1=0.0, scalar2=ccol[:, i:i + 1],
                                                    op0=AluOpType.max, op1=AluOpType.mult)
                            pv = psb.tile([128, 512], FP32, tag='psB')
                            for k in range(NK):
                                nc.tensor.matmul(pv, lhsT=xfm[:, k, i * 128:(i + 1) * 128],
                                                 rhs=wv_t[:, k, csl],
                                                 start=(k == 0), stop=(k == NK - 1))
                            if has_vb:
                                nc.vector.tensor_tensor(out=pv, in0=pv, in1=vbb[:, csl],
                                                        op=AluOpType.add)
                            nc.vector.tensor_copy(
                                out=v_aug[:, i, ch * 8:(ch + 1) * 8, 0:64],
                                in_=pv[:].rearrange('p (h d) -> p h d', d=64))

                # ============ Phase C: per-head kv partials + AllReduce =====
                kvp = bcp.tile([128, H * 65], FP32, tag='kvp')
                with tc.tile_pool(name='psC', bufs=8, space='PSUM') as psc:
                    for h in range(H):
                        pkv = psc.tile([128, 65], FP32, tag='psC')
                        hs = slice(h * 64, (h + 1) * 64)
                        for i in range(NT):
                            nc.tensor.matmul(pkv[0:64, :], lhsT=ks_all[:, i, hs],
                                             rhs=v_aug[:, i, h, :],
                                             start=(i == 0), stop=(i == NT - 1))
                            nc.tensor.matmul(pkv[64:128, :], lhsT=kc_all[:, i, hs],
                                             rhs=v_aug[:, i, h, :],
                                             tile_position=(0, 64),
                                             start=(i == 0), stop=(i == NT - 1))
                        nc.vector.tensor_copy(out=kvp[:, h * 65:(h + 1) * 65], in_=pkv)

                nc.gpsimd.dma_start(out=kv_cc_in[:], in_=kvp)
                nc.gpsimd.collective_compute(
                    'AllReduce', AluOpType.add,
                    ins=[kv_cc_in.opt()], outs=[kv_cc_out.opt()],
                    replica_groups=RG)

                with (
                    tc.tile_pool(name='de', bufs=1) as dep,
                ):
                    # kv back + to bf16
                    kvr = dep.tile([128, H * 65], FP32, tag='kvr')
                    nc.gpsimd.dma_start(out=kvr, in_=kv_cc_out[:])
                    kvb = dep.tile([128, H * 65], BF, tag='kvb')
                    nc.vector.tensor_copy(out=kvb, in_=kvr)

                    # qn transposes (feature-major bf16)
                    qnT = dep.tile([128, NJ, T], BF, tag='qnT')
                    for j in range(NJ):
                        nc.scalar.dma_start(out=qnT[:, j, :],
                                            in_=qn_dram[:, j * 128:(j + 1) * 128],
                                            transpose=True)

                    # ============ Phase D: q matmuls (feature-major out) ====
                    wq_t = wp.tile([128, NK, E], BF, tag='W')
                    nc.gpsimd.dma_start(out=wq_t,
                                        in_=d_wq[:].rearrange('(k p) e -> p k e', p=128))
                    qq = dep.tile([128, H, T], BF, tag='qq')
                    with tc.tile_pool(name='psD', bufs=4, space='PSUM') as psd:
                        for j in range(NJ):
                            for ch in range(2):
                                csl = slice(ch * 512, (ch + 1) * 512)
                                pq = psd.tile([128, 512], FP32, tag='psD')
                                for k in range(NK):
                                    nc.tensor.matmul(pq, lhsT=wq_t[:, k, j * 128:(j + 1) * 128],
                                                     rhs=qnT[:, k, csl],
                                                     start=(k == 0), stop=(k == NK - 1))
                                if has_qb:
                                    nc.vector.tensor_scalar(out=pq, in0=pq,
                                                            scalar1=qbc[:, j:j + 1],
                                                            scalar2=None,
                                                            op0=AluOpType.add)
                                # q_ tiles: head 2j from psum rows 0:64, 2j+1 from 64:128
                                for hh in range(2):
                                    h = 2 * j + hh
                                    rs = slice(hh * 64, (hh + 1) * 64)
                                    nc.vector.scalar_tensor_tensor(
                                        out=qq[0:64, h, csl], in0=pq[rs, :],
                                        scalar=0.0, in1=sbt[rs, csl],
                                        op0=AluOpType.max, op1=AluOpType.mult)
                                    nc.vector.scalar_tensor_tensor(
                                        out=qq[64:128, h, csl], in0=pq[rs, :],
                                        scalar=0.0, in1=cbt[rs, csl],
                                        op0=AluOpType.max, op1=AluOpType.mult)

                    # ============ Phase E+F: attention, residual, LN2 =======
                    with (
                        tc.tile_pool(name='ef', bufs=3) as efp,
                        tc.tile_pool(name='psE', bufs=4, space='PSUM') as pse,
                    ):
                        for i in range(NT):
                            rsl = slice(i * 128, (i + 1) * 128)
                            qnr = efp.tile([128, E], BF, tag='qnr')
                            nc.sync.dma_start(out=qnr, in_=qn_dram[rsl, :])
                            yt = efp.tile([128, E], FP32, tag='yt')
                            for h in range(H):
                                pa = pse.tile([128, 65], FP32, tag='psE')
                                nc.tensor.matmul(pa, lhsT=qq[:, h, rsl],
                                                 rhs=kvb[:, h * 65:(h + 1) * 65],
                                                 start=True, stop=True)
                                zt = efp.tile([128, 1], FP32, tag='zt')
                                nc.vector.tensor_scalar(out=zt, in0=pa[:, 64:65],
                                                        scalar1=EPS_ATTN, scalar2=None,
                                                        op0=AluOpType.max)
                                nc.vector.reciprocal(out=zt, in_=zt)
                                nc.vector.scalar_tensor_tensor(
                                    out=yt[:, h * 64:(h + 1) * 64], in0=pa[:, 0:64],
                                    scalar=zt, in1=qnr[:, h * 64:(h + 1) * 64],
                                    op0=AluOpType.mult, op1=AluOpType.add)
                            # LN2 on y tile
                            st2 = efp.tile([128, 2, 6], FP32, tag='st2')
                            yg = yt[:].rearrange('p (g d) -> p g d', g=2)
                            nc.vector.bn_stats(out=st2[:, 0, :], in_=yg[:, 0, :])
                            nc.vector.bn_stats(out=st2[:, 1, :], in_=yg[:, 1, :])
                            mv2 = efp.tile([128, 2], FP32, tag='mv2')
                            nc.vector.bn_aggr(out=mv2, in_=st2)
                            rstd2 = efp.tile([128, 1], FP32, tag='rstd2')
                            nc.scalar.activation(out=rstd2, in_=mv2[:, 1:2],
                                                 func=mybir.ActivationFunctionType.Sqrt,
                                                 bias=eps1, scale=1.0)
                            nc.vector.reciprocal(out=rstd2, in_=rstd2)
                            xh = efp.tile([128, E], BF, tag='xh')
                            nc.vector.tensor_scalar(out=xh, in0=yt, scalar1=mv2[:, 0:1],
                                                    scalar2=rstd2, op0=AluOpType.subtract,
                                                    op1=AluOpType.mult)
                            nc.gpsimd.dma_start(out=xh_dram[rsl, :], in_=xh)

            # ============ Phase G: output matmul ============================
            with (
                tc.tile_pool(name='gp', bufs=1) as gp,
                tc.tile_pool(name='go', bufs=3) as gop,
                tc.tile_pool(name='psG', bufs=4, space='PSUM') as psg,
            ):
                xhT = gp.tile([128, NJ, T], BF, tag='xhT')
                for j in range(NJ):
                    nc.scalar.dma_start(out=xhT[:, j, :],
                                        in_=xh_dram[:, j * 128:(j + 1) * 128],
                                        transpose=True)
                wo_t = wp.tile([128, NK, E], BF, tag='W')
                nc.gpsimd.dma_start(out=wo_t,
                                    in_=d_wo[:].rearrange('(k p) e -> p k e', p=128))
                for i in range(NT):
                    ot = gop.tile([128, E], FP32, tag='ot')
                    for ch in range(2):
                        csl = slice(ch * 512, (ch + 1) * 512)
                        po = psg.tile([128, 512], FP32, tag='psG')
                        for k in range(NK):
                            nc.tensor.matmul(po, lhsT=xhT[:, k, i * 128:(i + 1) * 128],
                                             rhs=wo_t[:, k, csl],
                                             start=(k == 0), stop=(k == NK - 1))
                        if has_b2o:
                            nc.vector.tensor_tensor(out=ot[:, csl], in0=po,
                                                    in1=b2ob[:, csl], op=AluOpType.add)
                        else:
                            nc.vector.tensor_copy(out=ot[:, csl], in_=po)
                    nc.sync.dma_start(out=d_out[i * 128:(i + 1) * 128, :], in_=ot)

    nc.compile()
    return nc


def _get_program(flags):
    if flags not in _BUILD_CACHE:
        _BUILD_CACHE[flags] = _build_program(flags)
    return _BUILD_CACHE[flags]


def _phm_weight(A, S):
    f = A.shape[0]
    din, dout = f * S.shape[1], f * S.shape[2]
    W = np.einsum('nij,nkl->ikjl', np.asarray(A, np.float32), np.asarray(S, np.float32))
    return np.ascontiguousarray(W.reshape(din, dout))


def kernel(**inputs):
    query = np.asarray(inputs['query'], np.float32)
    g1 = np.asarray(inputs['g1'], np.float32)
    b1 = np.asarray(inputs['b1'], np.float32)
    g2 = np.asarray(inputs['g2'], np.float32)
    b2 = np.asarray(inputs['b2'], np.float32)
    qb = np.asarray(inputs['qb'], np.float32)
    kb = np.asarray(inputs['kb'], np.float32)
    vb = np.asarray(inputs['vb'], np.float32)
    ob = np.asarray(inputs['ob'], np.float32)

    Wq = _phm_weight(inputs['qA'], inputs['qS'])
    Wk = _phm_weight(inputs['kA'], inputs['kS'])
    Wv = _phm_weight(inputs['vA'], inputs['vS'])
    Wo = _phm_weight(inputs['oA'], inputs['oS'])
    WoI = Wo + np.eye(E, dtype=np.float32)
    Wo2 = g2[:, None] * WoI
    B2O = b2 @ WoI + ob

    has_g1b1 = not (np.all(g1 == 1.0) and np.all(b1 == 0.0))
    has_qb = bool(np.any(qb != 0.0))
    has_kb = bool(np.any(kb != 0.0))
    has_vb = bool(np.any(vb != 0.0))
    has_b2o = bool(np.any(B2O != 0.0))
    flags = (has_g1b1, has_qb, has_kb, has_vb, has_b2o)

    nc = _get_program(flags)

    s_full = np.sin((np.pi / 2) * np.arange(1, L + 1, dtype=np.float32) / L)
    c_full = np.cos((np.pi / 2) * np.arange(1, L + 1, dtype=np.float32) / L)

    wq_b = Wq.astype(BF16)
    wk_b = Wk.astype(BF16)
    wv_b = Wv.astype(BF16)
    wo_b = Wo2.astype(BF16)

    in_maps = []
    for core in range(NCORES):
        b = core // 2
        l0 = (core % 2) * T
        x = np.ascontiguousarray(query[l0:l0 + T, b, :])
        s = s_full[l0:l0 + T]
        c = c_full[l0:l0 + T]
        im = {
            'x_tm': x,
            'x_fm': np.ascontiguousarray(x.T).astype(BF16),
            'wq': wq_b, 'wk': wk_b, 'wv': wv_b, 'wo2': wo_b,
            's_bcast': np.ascontiguousarray(np.broadcast_to(s, (128, T))).astype(BF16),
            'c_bcast': np.ascontiguousarray(np.broadcast_to(c, (128, T))).astype(BF16),
            's_cols': np.ascontiguousarray(s.reshape(NT, 128).T),
            'c_cols': np.ascontiguousarray(c.reshape(NT, 128).T),
        }
        if has_g1b1:
            im['g1_b'] = np.ascontiguousarray(np.broadcast_to(g1, (128, E)))
            im['b1_b'] = np.ascontiguousarray(np.broadcast_to(b1, (128, E)))
        if has_qb:
            im['qb_cols'] = np.ascontiguousarray(qb.reshape(NJ, 128).T)
        if has_kb:
            im['kb_b'] = np.ascontiguousarray(np.broadcast_to(kb, (128, E)))
        if has_vb:
            im['vb_b'] = np.ascontiguousarray(np.broadcast_to(vb, (128, E)))
        if has_b2o:
            im['b2o_b'] = np.ascontiguousarray(np.broadcast_to(B2O, (128, E)))
        in_maps.append(im)

    trace = bool(os.environ.get('KERNEL_TRACE'))
    res = run_bass_kernel_spmd(nc, in_maps, list(range(NCORES)), trace=trace)
    kernel._last_exec_ns = res.exec_time_ns

    out = np.empty((L, N, E), np.float32)
    for core in range(NCORES):
        b = core // 2
        l0 = (core % 2) * T
        out[l0:l0 + T, b, :] = res.results[core]['out']
    return out


kernel._last_exec_ns = None


# revision 4
# speedup vs baseline: 1.1780x; 1.1780x over previous
"""Cosformer attention Bass kernel for 8 trn2 NeuronCores.

Sharding: core c handles batch c//2, sequence half c%2 (1024 positions x 1
batch = 1024 tokens). Per-head linear-attention state (kv, ksum) is
AllReduce'd (bf16) between the two cores sharing a batch.

All matmuls run in bf16 with fp32 PSUM accumulation; LN stats and epilogues
in fp32. PHM weights are kron-expanded on host; LN2 affine + final residual
are folded into the output weight (Wo'' = diag(g2) @ (Wo + I)).
"""

import sys

for _p in ('/opt/trn_rl_repo',):
    if _p not in sys.path:
        sys.path.insert(0, _p)

import importlib.util as _ilu
import os

# The image's antenv lacks axon_hooks (needed for trace=True); register ours.
if 'antenv.axon_hooks' not in sys.modules:
    _hp = '/opt/trn_rl_repo/antenv/axon_hooks.py'
    if os.path.exists(_hp):
        _spec = _ilu.spec_from_file_location('antenv.axon_hooks', _hp)
        _mod = _ilu.module_from_spec(_spec)
        _spec.loader.exec_module(_mod)
        sys.modules['antenv.axon_hooks'] = _mod

import numpy as np
import ml_dtypes

import concourse.bass as bass
import concourse.tile as tile
from concourse import bacc, mybir
from concourse.alu_op_type import AluOpType
from concourse.bass_utils import run_bass_kernel_spmd

BF16 = ml_dtypes.bfloat16
FP32 = mybir.dt.float32
BF = mybir.dt.bfloat16
AF = mybir.ActivationFunctionType

L, N, E, H, D = 2048, 4, 1024, 16, 64
T = 1024            # tokens per core
NT = T // 128       # 8 token tiles
NK = E // 128       # 8 contraction tiles
NJ = E // 128       # 8 output-feature tiles
NCORES = 8
EPS_LN = 1e-5
EPS_ATTN = 1e-6

_BUILD_CACHE = {}


def _build_program(flags):
    """Build the SPMD Bass program. flags: (has_g1b1, has_qb, has_kb, has_vb, has_b2o)."""
    has_g1b1, has_qb, has_kb, has_vb, has_b2o = flags

    nc = bacc.Bacc("TRN2", target_bir_lowering=False, debug=False,
                   num_devices=NCORES)

    # ---- DRAM I/O ----
    d_x_tm = nc.dram_tensor('x_tm', [T, E], FP32, kind='ExternalInput')
    d_x_fm = nc.dram_tensor('x_fm', [E, T], BF, kind='ExternalInput')
    d_wq = nc.dram_tensor('wq', [E, E], BF, kind='ExternalInput')
    d_wk = nc.dram_tensor('wk', [E, E], BF, kind='ExternalInput')
    d_wv = nc.dram_tensor('wv', [E, E], BF, kind='ExternalInput')
    d_wo = nc.dram_tensor('wo2', [E, E], BF, kind='ExternalInput')
    d_sb = nc.dram_tensor('s_bcast', [128, T], BF, kind='ExternalInput')
    d_cb = nc.dram_tensor('c_bcast', [128, T], BF, kind='ExternalInput')
    d_scol = nc.dram_tensor('s_cols', [128, NT], FP32, kind='ExternalInput')
    d_ccol = nc.dram_tensor('c_cols', [128, NT], FP32, kind='ExternalInput')
    d_g1b = nc.dram_tensor('g1_b', [128, E], FP32, kind='ExternalInput') if has_g1b1 else None
    d_b1b = nc.dram_tensor('b1_b', [128, E], FP32, kind='ExternalInput') if has_g1b1 else None
    d_qbc = nc.dram_tensor('qb_cols', [128, NJ], FP32, kind='ExternalInput') if has_qb else None
    d_kbb = nc.dram_tensor('kb_b', [128, E], FP32, kind='ExternalInput') if has_kb else None
    d_vbb = nc.dram_tensor('vb_b', [128, E], FP32, kind='ExternalInput') if has_vb else None
    d_b2ob = nc.dram_tensor('b2o_b', [128, E], FP32, kind='ExternalInput') if has_b2o else None
    d_out = nc.dram_tensor('out', [T, E], FP32, kind='ExternalOutput')

    RG = [[0, 1], [2, 3], [4, 5], [6, 7]]

    with tile.TileContext(nc) as tc:
        with (
            tc.tile_pool(name='persist', bufs=1) as pp,
            tc.tile_pool(name='wpool', bufs=2) as wp,
            tc.tile_pool(name='dram', bufs=1, space='DRAM') as dp,
        ):
            # ---- constants (small, first on queues) ----
            sbt = pp.tile([128, T], BF, tag='sbt')
            cbt = pp.tile([128, T], BF, tag='cbt')
            scol = pp.tile([128, NT], FP32, tag='scol')
            ccol = pp.tile([128, NT], FP32, tag='ccol')
            eps1 = pp.tile([128, 1], FP32, tag='eps1')
            eps2 = pp.tile([128, 1], FP32, tag='eps2')
            nc.sync.dma_start(out=scol, in_=d_scol[:])
            nc.sync.dma_start(out=ccol, in_=d_ccol[:])
            nc.sync.dma_start(out=sbt, in_=d_sb[:])
            nc.sync.dma_start(out=cbt, in_=d_cb[:])
            nc.vector.memset(eps1, EPS_LN)
            nc.vector.memset(eps2, EPS_ATTN)
            g1b = b1b = qbc = kbb = vbb = b2ob = None
            if has_g1b1:
                g1b = pp.tile([128, E], FP32, tag='g1b')
                b1b = pp.tile([128, E], FP32, tag='b1b')
                nc.sync.dma_start(out=g1b, in_=d_g1b[:])
                nc.sync.dma_start(out=b1b, in_=d_b1b[:])
            if has_qb:
                qbc = pp.tile([128, NJ], FP32, tag='qbc')
                nc.sync.dma_start(out=qbc, in_=d_qbc[:])
            if has_kb:
                kbb = pp.tile([128, E], FP32, tag='kbb')
                nc.sync.dma_start(out=kbb, in_=d_kbb[:])
            if has_vb:
                vbb = pp.tile([128, E], FP32, tag='vbb')
                nc.sync.dma_start(out=vbb, in_=d_vbb[:])
            if has_b2o:
                b2ob = pp.tile([128, E], FP32, tag='b2ob')
                nc.sync.dma_start(out=b2ob, in_=d_b2ob[:])

            # persistent activation tiles
            qnT = pp.tile([128, NJ, T], BF, tag='qnT')     # qn feature-major
            kvb = pp.tile([128, H * 65], BF, tag='kvb')    # reduced kv (bf16)
            qq = pp.tile([128, H, T], BF, tag='qq')        # q_ per head, fm

            # DRAM scratch
            qn_dram = dp.tile([T, E], BF)
            xh_dram = dp.tile([T, E], BF)
            kv_cc_in = dp.tile([128, H * 65], BF)
            kv_cc_out = dp.tile([128, H * 65], BF)

            with (
                tc.tile_pool(name='xfmp', bufs=1) as xfmp,
                tc.tile_pool(name='bc', bufs=1) as bcp,
                tc.tile_pool(name='ln1', bufs=3) as ap,
            ):
                # feature-major x: split DMA, sync queue (highest priority)
                xfm = xfmp.tile([128, NK, T], BF, tag='xfm')
                xfm_src = d_x_fm[:].rearrange('(k p) t -> p k t', p=128)
                nc.sync.dma_start(out=xfm[:, 0:4, :], in_=xfm_src[:, 0:4, :])
                nc.sync.dma_start(out=xfm[:, 4:8, :], in_=xfm_src[:, 4:8, :])

                # weights: Wk on scalar queue, Wv on gpsimd, both split
                wk_t = wp.tile([128, NK, E], BF, tag='W')
                wk_src = d_wk[:].rearrange('(k p) e -> p k e', p=128)
                nc.scalar.dma_start(out=wk_t[:, 0:4, :], in_=wk_src[:, 0:4, :])
                nc.scalar.dma_start(out=wk_t[:, 4:8, :], in_=wk_src[:, 4:8, :])
                wv_t = wp.tile([128, NK, E], BF, tag='W')
                wv_src = d_wv[:].rearrange('(k p) e -> p k e', p=128)
                nc.gpsimd.dma_start(out=wv_t[:, 0:4, :], in_=wv_src[:, 0:4, :])
                nc.gpsimd.dma_start(out=wv_t[:, 4:8, :], in_=wv_src[:, 4:8, :])

                # [ks | kc] interleaved per head: kv needs single (128,128) lhsT
                ksc = bcp.tile([128, NT, H, 128], BF, tag='ksc')
                v_aug = bcp.tile([128, NT, H, 65], BF, tag='vaug')
                kvp = bcp.tile([128, H * 65], BF, tag='kvp')
                nc.vector.memset(v_aug[:, :, :, 64:65], 1.0)

                # ============ Phase B1: k matmuls ============
                with tc.tile_pool(name='psB', bufs=4, space='PSUM') as psb:
                    for i in range(NT):
                        for ch in range(2):
                            csl = slice(ch * 512, (ch + 1) * 512)
                            pk = psb.tile([128, 512], FP32, tag='psB')
                            for k in range(NK):
                                nc.tensor.matmul(pk, lhsT=xfm[:, k, i * 128:(i + 1) * 128],
                                                 rhs=wk_t[:, k, csl],
                                                 start=(k == 0), stop=(k == NK - 1))
                            if has_kb:
                                nc.vector.tensor_tensor(out=pk, in0=pk, in1=kbb[:, csl],
                                                        op=AluOpType.add)
                            pkv = pk[:].rearrange('p (h d) -> p h d', d=64)
                            # relu+scale on ACT: Relu(psum * s) == relu(psum)*s (s>0)
                            nc.scalar.activation(
                                out=ksc[:, i, ch * 8:(ch + 1) * 8, 0:64], in_=pkv,
                                func=AF.Relu, scale=scol[:, i:i + 1])
                            nc.scalar.activation(
                                out=ksc[:, i, ch * 8:(ch + 1) * 8, 64:128], in_=pkv,
                                func=AF.Relu, scale=ccol[:, i:i + 1])

                    # ============ Phase A: LN1 -> qn (bf16) to DRAM =========
                    for i in range(NT):
                        xt = ap.tile([128, E], FP32, tag='xt')
                        nc.sync.dma_start(out=xt, in_=d_x_tm[i * 128:(i + 1) * 128, :])
                        st = ap.tile([128, 2, 6], FP32, tag='st')
                        xg = xt[:].rearrange('p (g d) -> p g d', g=2)
                        nc.vector.bn_stats(out=st[:, 0, :], in_=xg[:, 0, :])
                        nc.vector.bn_stats(out=st[:, 1, :], in_=xg[:, 1, :])
                        mv = ap.tile([128, 2], FP32, tag='mv')
                        nc.vector.bn_aggr(out=mv, in_=st)
                        rstd = ap.tile([128, 1], FP32, tag='rstd')
                        nc.scalar.activation(out=rstd, in_=mv[:, 1:2], func=AF.Sqrt,
                                             bias=eps1, scale=1.0)
                        nc.vector.reciprocal(out=rstd, in_=rstd)
                        qnt = ap.tile([128, E], BF, tag='qnt')
                        if has_g1b1:
                            tmp = ap.tile([128, E], FP32, tag='qtmp')
                            nc.vector.tensor_scalar(out=tmp, in0=xt, scalar1=mv[:, 0:1],
                                                    scalar2=rstd, op0=AluOpType.subtract,
                                                    op1=AluOpType.mult)
                            nc.vector.tensor_mul(tmp, tmp, g1b)
                            nc.vector.tensor_tensor(out=qnt, in0=tmp, in1=b1b,
                                                    op=AluOpType.add)
                        else:
                            nc.vector.tensor_scalar(out=qnt, in0=xt, scalar1=mv[:, 0:1],
                                                    scalar2=rstd, op0=AluOpType.subtract,
                                                    op1=AluOpType.mult)
                        nc.gpsimd.dma_start(out=qn_dram[i * 128:(i + 1) * 128, :], in_=qnt)

                    # qn transposes (feature-major); scalar queue (xbar)
                    for j in range(NJ):
                        nc.scalar.dma_start(out=qnT[:, j, :],
                                            in_=qn_dram[:, j * 128:(j + 1) * 128],
                                            transpose=True)

                    # ============ Phase B2: v matmuls ============
                    for i in range(NT):
                        for ch in range(2):
                            csl = slice(ch * 512, (ch + 1) * 512)
                            pv = psb.tile([128, 512], FP32, tag='psB')
                            for k in range(NK):
                                nc.tensor.matmul(pv, lhsT=xfm[:, k, i * 128:(i + 1) * 128],
                                                 rhs=wv_t[:, k, csl],
                                                 start=(k == 0), stop=(k == NK - 1))
                            if has_vb:
                                nc.vector.tensor_tensor(out=pv, in0=pv, in1=vbb[:, csl],
                                                        op=AluOpType.add)
                            nc.vector.tensor_copy(
                                out=v_aug[:, i, ch * 8:(ch + 1) * 8, 0:64],
                                in_=pv[:].rearrange('p (h d) -> p h d', d=64))

                # Wq load early (Wk slot frees after B1)
                wq_t = wp.tile([128, NK, E], BF, tag='W')
                wq_src = d_wq[:].rearrange('(k p) e -> p k e', p=128)
                nc.gpsimd.dma_start(out=wq_t[:, 0:4, :], in_=wq_src[:, 0:4, :])
                nc.gpsimd.dma_start(out=wq_t[:, 4:8, :], in_=wq_src[:, 4:8, :])

                # ============ Phase C: per-head kv partials + AllReduce =====
                with tc.tile_pool(name='psC', bufs=8, space='PSUM') as psc:
                    for h in range(H):
                        pkv = psc.tile([128, 65], FP32, tag='psC')
                        for i in range(NT):
                            nc.tensor.matmul(pkv, lhsT=ksc[:, i, h, :],
                                             rhs=v_aug[:, i, h, :],
                                             start=(i == 0), stop=(i == NT - 1))
                        nc.vector.tensor_copy(out=kvp[:, h * 65:(h + 1) * 65], in_=pkv)

                nc.gpsimd.dma_start(out=kv_cc_in[:], in_=kvp)
                nc.gpsimd.collective_compute(
                    'AllReduce', AluOpType.add,
                    ins=[kv_cc_in.opt()], outs=[kv_cc_out.opt()],
                    replica_groups=RG)
                nc.gpsimd.dma_start(out=kvb, in_=kv_cc_out[:])

            # ============ Phase D: q matmuls (feature-major out) ============
            with tc.tile_pool(name='psD', bufs=4, space='PSUM') as psd:
                for j in range(NJ):
                    for ch in range(2):
                        csl = slice(ch * 512, (ch + 1) * 512)
                        pq = psd.tile([128, 512], FP32, tag='psD')
                        for k in range(NK):
                            nc.tensor.matmul(pq, lhsT=wq_t[:, k, j * 128:(j + 1) * 128],
                                             rhs=qnT[:, k, csl],
                                             start=(k == 0), stop=(k == NK - 1))
                        if has_qb:
                            nc.vector.tensor_scalar(out=pq, in0=pq,
                                                    scalar1=qbc[:, j:j + 1],
                                                    scalar2=None, op0=AluOpType.add)
                        # q_ tiles: head 2j from psum rows 0:64, 2j+1 from 64:128
                        for hh in range(2):
                            h = 2 * j + hh
                            rs = slice(hh * 64, (hh + 1) * 64)
                            nc.vector.scalar_tensor_tensor(
                                out=qq[0:64, h, csl], in0=pq[rs, :],
                                scalar=0.0, in1=sbt[rs, csl],
                                op0=AluOpType.max, op1=AluOpType.mult)
                            nc.vector.scalar_tensor_tensor(
                                out=qq[64:128, h, csl], in0=pq[rs, :],
                                scalar=0.0, in1=cbt[rs, csl],
                                op0=AluOpType.max, op1=AluOpType.mult)

            # Wo2 load (Wv slot frees after B2)
            wo_t = wp.tile([128, NK, E], BF, tag='W')
            wo_src = d_wo[:].rearrange('(k p) e -> p k e', p=128)
            nc.gpsimd.dma_start(out=wo_t[:, 0:4, :], in_=wo_src[:, 0:4, :])
            nc.gpsimd.dma_start(out=wo_t[:, 4:8, :], in_=wo_src[:, 4:8, :])

            # ============ Phase E+F: attention, residual, LN2 ===============
            with (
                tc.tile_pool(name='gx', bufs=1) as gxp,
                tc.tile_pool(name='ef', bufs=3) as efp,
                tc.tile_pool(name='psE', bufs=4, space='PSUM') as pse,
            ):
                xhT = gxp.tile([128, NJ, T], BF, tag='xhT')

                def emit_xh_transpose(half):
                    tsl = slice(half * 512, (half + 1) * 512)
                    for j in range(NJ):
                        nc.scalar.dma_start(out=xhT[:, j, tsl],
                                            in_=xh_dram[tsl, j * 128:(j + 1) * 128],
                                            transpose=True)

                for i in range(NT):
                    rsl = slice(i * 128, (i + 1) * 128)
                    qnr = efp.tile([128, E], BF, tag='qnr')
                    nc.sync.dma_start(out=qnr, in_=qn_dram[rsl, :])
                    yt = efp.tile([128, H, 64], FP32, tag='yt')
                    dcol = efp.tile([128, H], FP32, tag='dcol')
                    z16 = efp.tile([128, H], FP32, tag='z16')
                    pas = []
                    for g in range(4):
                        pa = pse.tile([128, 4 * 65], FP32, tag='psE')
                        pas.append(pa)
                        for hh in range(4):
                            h = 4 * g + hh
                            nc.tensor.matmul(pa[:, hh * 65:(hh + 1) * 65],
                                             lhsT=qq[:, h, rsl],
                                             rhs=kvb[:, h * 65:(h + 1) * 65],
                                             start=True, stop=True)
                        pav = pa[:].rearrange('p (h c) -> p h c', c=65)
                        nc.vector.tensor_copy(out=dcol[:, g * 4:(g + 1) * 4],
                                              in_=pav[:, :, 64])
                    # z = 1/max(denom, eps), batched over all heads
                    nc.vector.tensor_scalar(out=z16, in0=dcol, scalar1=EPS_ATTN,
                                            scalar2=None, op0=AluOpType.max)
                    nc.vector.reciprocal(out=z16, in_=z16)
                    for g in range(4):
                        pav = pas[g][:].rearrange('p (h c) -> p h c', c=65)
                        zb = z16[:, g * 4:(g + 1) * 4].broadcast_to((128, 4, 64))
                        nc.vector.tensor_tensor(out=yt[:, g * 4:(g + 1) * 4, :],
                                                in0=pav[:, :, 0:64], in1=zb,
                                                op=AluOpType.mult)
                    ytf = yt[:].rearrange('p h d -> p (h d)')
                    nc.vector.tensor_tensor(out=ytf, in0=ytf, in1=qnr,
                                            op=AluOpType.add)
                    # LN2
                    st2 = efp.tile([128, 2, 6], FP32, tag='st2')
                    yg = yt[:].rearrange('p (g x) d -> p g (x d)', g=2)
                    nc.vector.bn_stats(out=st2[:, 0, :], in_=yg[:, 0, :])
                    nc.vector.bn_stats(out=st2[:, 1, :], in_=yg[:, 1, :])
                    mv2 = efp.tile([128, 2], FP32, tag='mv2')
                    nc.vector.bn_aggr(out=mv2, in_=st2)
                    rstd2 = efp.tile([128, 1], FP32, tag='rstd2')
                    nc.scalar.activation(out=rstd2, in_=mv2[:, 1:2], func=AF.Sqrt,
                                         bias=eps1, scale=1.0)
                    nc.vector.reciprocal(out=rstd2, in_=rstd2)
                    xh = efp.tile([128, E], BF, tag='xh')
                    nc.vector.tensor_scalar(out=xh, in0=ytf, scalar1=mv2[:, 0:1],
                                            scalar2=rstd2, op0=AluOpType.subtract,
                                            op1=AluOpType.mult)
                    nc.gpsimd.dma_start(out=xh_dram[rsl, :], in_=xh)
                    if i == 3:
                        emit_xh_transpose(0)
                emit_xh_transpose(1)

                # ============ Phase G: output matmul ========================
                with (
                    tc.tile_pool(name='go', bufs=3) as gop,
                    tc.tile_pool(name='psG', bufs=4, space='PSUM') as psg,
                ):
                    for i in range(NT):
                        for ch in range(2):
                            csl = slice(ch * 512, (ch + 1) * 512)
                            po = psg.tile([128, 512], FP32, tag='psG')
                            for k in range(NK):
                                nc.tensor.matmul(po, lhsT=xhT[:, k, i * 128:(i + 1) * 128],
                                                 rhs=wo_t[:, k, csl],
                                                 start=(k == 0), stop=(k == NK - 1))
                            ot = gop.tile([128, 512], FP32, tag='ot')
                            if has_b2o:
                                nc.vector.tensor_tensor(out=ot, in0=po,
                                                        in1=b2ob[:, csl], op=AluOpType.add)
                            else:
                                nc.vector.tensor_copy(out=ot, in_=po)
                            nc.sync.dma_start(out=d_out[i * 128:(i + 1) * 128, csl],
                                              in_=ot)

    nc.compile()
    return nc


def _get_program(flags):
    if flags not in _BUILD_CACHE:
        _BUILD_CACHE[flags] = _build_program(flags)
    return _BUILD_CACHE[flags]


def _phm_weight(A, S):
    f = A.shape[0]
    din, dout = f * S.shape[1], f * S.shape[2]
    W = np.einsum('nij,nkl->ikjl', np.asarray(A, np.float32), np.asarray(S, np.float32))
    return np.ascontiguousarray(W.reshape(din, dout))


def kernel(**inputs):
    query = np.asarray(inputs['query'], np.float32)
    g1 = np.asarray(inputs['g1'], np.float32)
    b1 = np.asarray(inputs['b1'], np.float32)
    g2 = np.asarray(inputs['g2'], np.float32)
    b2 = np.asarray(inputs['b2'], np.float32)
    qb = np.asarray(inputs['qb'], np.float32)
    kb = np.asarray(inputs['kb'], np.float32)
    vb = np.asarray(inputs['vb'], np.float32)
    ob = np.asarray(inputs['ob'], np.float32)

    Wq = _phm_weight(inputs['qA'], inputs['qS'])
    Wk = _phm_weight(inputs['kA'], inputs['kS'])
    Wv = _phm_weight(inputs['vA'], inputs['vS'])
    Wo = _phm_weight(inputs['oA'], inputs['oS'])
    WoI = Wo + np.eye(E, dtype=np.float32)
    Wo2 = g2[:, None] * WoI
    B2O = b2 @ WoI + ob

    has_g1b1 = not (np.all(g1 == 1.0) and np.all(b1 == 0.0))
    has_qb = bool(np.any(qb != 0.0))
    has_kb = bool(np.any(kb != 0.0))
    has_vb = bool(np.any(vb != 0.0))
    has_b2o = bool(np.any(B2O != 0.0))
    flags = (has_g1b1, has_qb, has_kb, has_vb, has_b2o)

    nc = _get_program(flags)

    s_full = np.sin((np.pi / 2) * np.arange(1, L + 1, dtype=np.float32) / L)
    c_full = np.cos((np.pi / 2) * np.arange(1, L + 1, dtype=np.float32) / L)

    wq_b = Wq.astype(BF16)
    wk_b = Wk.astype(BF16)
    wv_b = Wv.astype(BF16)
    wo_b = Wo2.astype(BF16)

    in_maps = []
    for core in range(NCORES):
        b = core // 2
        l0 = (core % 2) * T
        x = np.ascontiguousarray(query[l0:l0 + T, b, :])
        s = s_full[l0:l0 + T]
        c = c_full[l0:l0 + T]
        im = {
            'x_tm': x,
            'x_fm': np.ascontiguousarray(x.T).astype(BF16),
            'wq': wq_b, 'wk': wk_b, 'wv': wv_b, 'wo2': wo_b,
            's_bcast': np.ascontiguousarray(np.broadcast_to(s, (128, T))).astype(BF16),
            'c_bcast': np.ascontiguousarray(np.broadcast_to(c, (128, T))).astype(BF16),
            's_cols': np.ascontiguousarray(s.reshape(NT, 128).T),
            'c_cols': np.ascontiguousarray(c.reshape(NT, 128).T),
        }
        if has_g1b1:
            im['g1_b'] = np.ascontiguousarray(np.broadcast_to(g1, (128, E)))
            im['b1_b'] = np.ascontiguousarray(np.broadcast_to(b1, (128, E)))
        if has_qb:
            im['qb_cols'] = np.ascontiguousarray(qb.reshape(NJ, 128).T)
        if has_kb:
            im['kb_b'] = np.ascontiguousarray(np.broadcast_to(kb, (128, E)))
        if has_vb:
            im['vb_b'] = np.ascontiguousarray(np.broadcast_to(vb, (128, E)))
        if has_b2o:
            im['b2o_b'] = np.ascontiguousarray(np.broadcast_to(B2O, (128, E)))
        in_maps.append(im)

    trace = bool(os.environ.get('KERNEL_TRACE'))
    res = run_bass_kernel_spmd(nc, in_maps, list(range(NCORES)), trace=trace)
    kernel._last_exec_ns = res.exec_time_ns

    out = np.empty((L, N, E), np.float32)
    for core in range(NCORES):
        b = core // 2
        l0 = (core % 2) * T
        out[l0:l0 + T, b, :] = res.results[core]['out']
    return out


kernel._last_exec_ns = None
